# revision 2
# baseline (speedup 1.0000x reference)
"""GNN message passing (3x GraphConv+BN+ReLU, final GraphConv) on 8 trn2 cores.

v2: source-sharded partial aggregation + ReduceScatter.
  - Nodes partitioned 8 cores x 51 chunks x 128 slots (N_PAD=52224), with a
    two-phase balance so every (src core, dst chunk) has <=256 edges ->
    uniform 2 tiles of 128 edges per chunk, minimal padding.
  - Each core keeps a LOCAL node-major fp16 table of its own nodes; per layer
    it gathers its edges' source rows (indirect DMA), segment-sums them into
    partial aggregates for ALL 408 dst chunks via one-hot matmuls in PSUM,
    writes fp16 partials to DRAM, and a ReduceScatter(add) delivers each
    core's own aggregated chunks (output 8x smaller than an AllGather).
  - GraphConv bias is absorbed by training-mode BatchNorm (shift-invariant)
    and dropped for inner layers; BN stats use a N_PAD/N correction with pad
    slots pinned to exact zero (mask folded into the transpose copy).
  - Final layer transforms partials by Wrel2 BEFORE the ReduceScatter, so the
    last collective output is only [2, 6528].
"""

import sys

import numpy as np

sys.path.insert(0, "/opt/trn_rl_repo")

import concourse.bass as bass  # noqa: E402
import concourse.mybir as mybir  # noqa: E402
import concourse.tile as tile  # noqa: E402
from concourse.vector_clock import ScopedClock  # noqa: E402
from concourse import library_config  # noqa: E402
from concourse.library_overlay import lower_extended_insts  # noqa: E402

N = 50000
E = 800000
D = 128
L = 3
OUT = 2
EPS = 1e-5
N_CORES = 8
P = 128
CHUNKS = 51                      # dst chunks per core
C_TOT = N_CORES * CHUNKS         # 408
SLOTS = CHUNKS * P               # 6528 per core
N_PAD = N_CORES * SLOTS          # 52224
KT = 2                           # edge tiles per chunk
TILES = C_TOT * KT               # 816 tiles per core per layer
NIDX = TILES * P                 # 104448 gathered rows per core per layer
CPG = 12                         # chunks per gather call (408 = 34*12)
CALLS = C_TOT // CPG             # 34
BANKG = 3                        # chunks per PSUM bank flush (51 = 17*3)
ZBLK = 4                         # own chunks per dense-z matmul block
TBLK = 17                        # chunks per table-rebuild block (51 = 3*17)
PA = 36                          # piece-A chunks per core (processed first)
PB = CHUNKS - PA                 # 15; piece B reduced right after piece A
STATC = 52224.0 / 50000.0        # pad-slot correction for BN stats

F16 = mybir.dt.float16
F32 = mybir.dt.float32

# ---------------------------------------------------------------------------
# walrus in this container accepts at most ONE semaphore wait per instruction.
# Patch the Tile exit drain and add a post-pass splitting multi-wait insts.
# ---------------------------------------------------------------------------
_MAX_WAITS = 1


def _drain_and_barrier(self, tick_clock, wait_clock):
    nc = self.nc
    drain_inst = nc.sync.drain()
    wait_clock.add_sem_waits(
        drain_inst.ins, ScopedClock({None: tick_clock.global_clock})
    )
    si = drain_inst.ins.sync_info
    if si is not None and si.on_wait is not None and len(si.on_wait) > _MAX_WAITS:
        waits = list(si.on_wait)
        si.on_wait = waits[:_MAX_WAITS]
        rest = waits[_MAX_WAITS:]
        for i in range(0, len(rest), _MAX_WAITS):
            nop = nc.sync.nop(nofuse=True)
            nop.ins.sync_info = mybir.SyncInfo(
                on_wait=rest[i : i + _MAX_WAITS], on_update=[]
            )
    nc.all_engine_barrier()
    assert self.sems is not None
    popped = nc._tile_sem_poison_stack.pop()
    assert popped is self._sem_poison
    nc.clear_and_free_semaphores(list(self.sems.allocated().values()))
    nc.all_engine_barrier()


tile.TileContext._drain_and_barrier = _drain_and_barrier


def _split_multiwait(nc):
    n_split = 0
    for fn in nc.m.functions:
        for blk in fn.blocks:
            out = []
            for inst in blk.instructions:
                si = inst.sync_info
                if si is not None and si.on_wait and len(si.on_wait) > _MAX_WAITS:
                    waits = list(si.on_wait)
                    si.on_wait = waits[-_MAX_WAITS:]
                    rest = waits[:-_MAX_WAITS]
                    for i in range(0, len(rest), _MAX_WAITS):
                        n_split += 1
                        out.append(
                            mybir.InstNoOp(
                                name=f"{inst.name}-ws{i}",
                                engine=inst.engine,
                                ins=[],
                                outs=[],
                                bass_nofuse=True,
                                sync_info=mybir.SyncInfo(
                                    on_wait=rest[i : i + _MAX_WAITS], on_update=[]
                                ),
                                debug=inst.debug,
                            )
                        )
                out.append(inst)
            blk.instructions[:] = out
    return n_split


# ---------------------------------------------------------------------------
# Host-side graph partitioning
# ---------------------------------------------------------------------------
def _partition(src, dst):
    """Two-phase partition: nodes -> cores (phase 1), then per core nodes ->
    51 chunks (phase 2) balancing the 8-dim in-edges-by-src-core vector with
    hard cap 256 per (src core, chunk)."""
    deg_out = np.bincount(src, minlength=N)
    deg_in = np.bincount(dst, minlength=N)

    order = np.argsort(-(deg_in + deg_out), kind="stable")
    core_of = np.empty(N, np.int32)
    tot = np.zeros(N_CORES)
    cnt = np.zeros(N_CORES, np.int64)
    cap_nodes = CHUNKS * P
    for nd in order:
        score = tot + 1e12 * (cnt >= cap_nodes)
        c = int(np.argmin(score))
        core_of[nd] = c
        tot[c] += deg_in[nd] + deg_out[nd]
        cnt[c] += 1

    vec = np.zeros((N, N_CORES), np.int32)
    np.add.at(vec, (dst, core_of[src]), 1)

    chunk_of = np.full(N, -1, np.int32)
    slot_of = np.full(N, -1, np.int32)
    cap_edge = KT * P
    max_cell = 0
    for k in range(N_CORES):
        nodes = np.where(core_of == k)[0]
        v = vec[nodes]
        o = np.argsort(-v.sum(1), kind="stable")
        fill = np.zeros((CHUNKS, N_CORES), np.int64)
        nfill = np.zeros(CHUNKS, np.int64)
        for i in o:
            w = v[i]
            res = fill + w[None, :]
            over = np.maximum(res - cap_edge, 0).sum(1)
            score = over * 100000 + res.max(1) * 100 + nfill
            score[nfill >= P] = 1 << 60
            g = int(np.argmin(score))
            nd = nodes[i]
            chunk_of[nd] = g
            slot_of[nd] = nfill[g]
            fill[g] += w
            nfill[g] += 1
        max_cell = max(max_cell, int(fill.max()))
    return core_of, chunk_of, slot_of, max_cell


def _preprocess(x, edge_index):
    x = np.asarray(x, np.float32)
    ei = np.asarray(edge_index)
    src = ei[0].astype(np.int64)
    dst = ei[1].astype(np.int64)

    core_of, chunk_of, slot_of, max_cell = _partition(src, dst)
    spill = max_cell > KT * P  # overflow fallback: caller bumps balancing
    assert not spill, f"partition overflow: max cell {max_cell}"

    newid = (
        core_of.astype(np.int64) * SLOTS
        + chunk_of.astype(np.int64) * P
        + slot_of
    )

    # Edge streams per owning (source) core, ordered by PROCESSING position:
    # piece A (chunks 0..PA-1 of each core, core-major) first, then piece B.
    g2proc = np.empty(C_TOT, np.int64)
    pos_i = 0
    for k in range(N_CORES):
        g2proc[k * CHUNKS : k * CHUNKS + PA] = np.arange(pos_i, pos_i + PA)
        pos_i += PA
    for k in range(N_CORES):
        g2proc[k * CHUNKS + PA : (k + 1) * CHUNKS] = np.arange(pos_i, pos_i + PB)
        pos_i += PB

    e_core = core_of[src]
    gchunk = g2proc[core_of[dst].astype(np.int64) * CHUNKS + chunk_of[dst]]
    loc_src = (newid[src] - e_core.astype(np.int64) * SLOTS).astype(np.int16)
    rel_dst = slot_of[dst].astype(np.int16)

    idx_cores = np.zeros((N_CORES, P, NIDX // 16), np.int16)
    dstv_cores = np.full((N_CORES, P, TILES), -1.0, np.float16)
    for c in range(N_CORES):
        m = e_core == c
        gc = gchunk[m]
        o = np.argsort(gc, kind="stable")
        gc = gc[o]
        ls = loc_src[m][o]
        rd = rel_dst[m][o]
        counts = np.bincount(gc, minlength=C_TOT)
        assert counts.max() <= KT * P
        starts = np.concatenate([[0], np.cumsum(counts)[:-1]])
        pos = np.arange(len(gc)) - starts[gc]
        flat = gc * (KT * P) + pos  # slot in the padded edge stream
        idx_full = np.zeros(NIDX, np.int16)
        idx_full[flat] = ls
        dv_full = np.full(NIDX, -1.0, np.float16)
        dv_full[flat] = rd.astype(np.float16)
        # gather idx wrap: logical i -> [i % 16, i // 16], replicated to 128
        wrapped = idx_full.reshape(NIDX // 16, 16).T  # [16, NIDX//16]
        idx_cores[c] = np.tile(wrapped, (8, 1))
        # dst values: edge slot (tile, p) -> dstv[p, tile]
        dstv_cores[c] = dv_full.reshape(TILES, P).T

    # local node tables
    x_pad = np.zeros((N_PAD, D), np.float32)
    x_pad[newid] = x
    x_loc = np.ascontiguousarray(
        x_pad.reshape(N_CORES, SLOTS, D).astype(np.float16)
    )
    xT_loc = np.ascontiguousarray(
        x_pad.reshape(N_CORES, SLOTS, D).transpose(0, 2, 1).astype(np.float16)
    )

    # pad-slot masks: [core, 1, SLOTS] fp16 rows, 1.0 for real nodes
    mask = np.zeros((N_CORES, 1, SLOTS), np.float16)
    sl = newid % SLOTS
    mask[core_of, 0, sl] = 1.0
    mask = np.broadcast_to(mask, (N_CORES, P, SLOTS))

    return newid, idx_cores, dstv_cores, x_loc, xT_loc, mask


# ---------------------------------------------------------------------------
# Device program
# ---------------------------------------------------------------------------
def build_program():
    nc = bass.Bass(num_devices=N_CORES)

    p_xloc = nc.declare_dram_parameter("x_loc", [SLOTS, D], F16, isOutput=False)
    p_xT = nc.declare_dram_parameter("xT_loc", [D, SLOTS], F16, isOutput=False)
    p_idx = nc.declare_dram_parameter("gidx", [P, NIDX // 16], mybir.dt.int16, isOutput=False)
    p_dstv = nc.declare_dram_parameter("dstv", [P, TILES], F16, isOutput=False)
    p_wrel = nc.declare_dram_parameter("wrel", [L, D, D], F16, isOutput=False)
    p_wroot = nc.declare_dram_parameter("wroot", [L, D, D], F16, isOutput=False)
    p_w2 = nc.declare_dram_parameter("w2", [D, 2 * OUT], F16, isOutput=False)
    p_gammaT = nc.declare_dram_parameter("gammaT", [D, L], F32, isOutput=False)
    p_betaT = nc.declare_dram_parameter("betaT", [D, L], F32, isOutput=False)
    p_b2 = nc.declare_dram_parameter("b2", [OUT, 1], F32, isOutput=False)
    p_iotar = nc.declare_dram_parameter("iotar", [P, P * KT], F16, isOutput=False)
    p_mask = nc.declare_dram_parameter("mask16", [P, SLOTS], F16, isOutput=False)
    p_out = nc.declare_dram_parameter("z4T", [OUT, SLOTS], F32, isOutput=True)

    rg = [list(range(N_CORES))]
    ICALL = CPG * KT * P // 16   # idx columns per gather call (384)

    with tile.TileContext(nc) as tc:
        with (
            tc.tile_pool(name="dram_tab", bufs=2, space="DRAM") as dram_tab,
            tc.tile_pool(name="dram_rs", bufs=2, space="DRAM") as dram_rs,
            tc.tile_pool(name="dram_cc", bufs=2, space="DRAM") as dram_cc,
            tc.tile_pool(name="singles", bufs=1) as singles,
            tc.tile_pool(name="hT", bufs=2) as hT_pool,
            tc.tile_pool(name="zb", bufs=1) as z_pool,
            tc.tile_pool(name="agg", bufs=1) as agg_pool,
            tc.tile_pool(name="gath", bufs=3) as g_pool,
            tc.tile_pool(name="sel", bufs=2) as s_pool,
            tc.tile_pool(name="aggst", bufs=3) as st_pool,
            tc.tile_pool(name="z2st", bufs=1) as z2_pool,
            tc.tile_pool(name="t16p", bufs=2) as t16_pool,
            tc.tile_pool(name="bns", bufs=2) as bn_pool,
            tc.tile_pool(name="stat", bufs=2) as stat_pool,
            tc.tile_pool(name="psA", bufs=4, space="PSUM") as psA,
            tc.tile_pool(name="psZ", bufs=2, space="PSUM") as psZ,
        ):
            with tc.high_priority():
                nc.gpsimd.load_library(library_config.mlp)
            nidx_reg = nc.gpsimd.to_reg(CPG * KT * P)

            # --- static data in SBUF ---
            idx_sb = singles.tile([P, NIDX // 16], mybir.dt.int16)
            nc.sync.dma_start(out=idx_sb[:], in_=p_idx[:])
            dstv_sb = singles.tile([P, TILES], F16)
            nc.sync.dma_start(out=dstv_sb[:], in_=p_dstv[:])
            iotar_sb = singles.tile([P, P * KT], F16)
            nc.sync.dma_start(out=iotar_sb[:], in_=p_iotar[:])
            mask_sb = singles.tile([P, SLOTS], F16)
            nc.sync.dma_start(out=mask_sb[:], in_=p_mask[:])
            wrel_sb = singles.tile([P, L * D], F16)
            wroot_sb = singles.tile([P, L * D], F16)
            for l in range(L):
                nc.sync.dma_start(out=wrel_sb[:, l * D : (l + 1) * D], in_=p_wrel[l])
                nc.sync.dma_start(out=wroot_sb[:, l * D : (l + 1) * D], in_=p_wroot[l])
            w2_sb = singles.tile([P, 2 * OUT], F16)
            nc.sync.dma_start(out=w2_sb[:], in_=p_w2[:])
            gammaT_sb = singles.tile([P, L], F32)
            nc.sync.dma_start(out=gammaT_sb[:], in_=p_gammaT[:])
            betaT_sb = singles.tile([P, L], F32)
            nc.sync.dma_start(out=betaT_sb[:], in_=p_betaT[:])
            b2_sb = singles.tile([OUT, 1], F32)
            nc.sync.dma_start(out=b2_sb[:], in_=p_b2[:])
            eps_sb = singles.tile([P, 1], F32)
            nc.vector.memset(eps_sb[:], EPS)

            hT_prev = hT_pool.tile([P, SLOTS], F16, tag="hT")
            nc.sync.dma_start(out=hT_prev[:], in_=p_xT[:])
            h_tab = None  # layer 0 gathers read p_xloc

            out_sb = singles.tile([OUT, SLOTS], F32)

            for l in range(L + 1):
                is_final = l == L

                # ------- partial aggregation over all 408 chunks, split into
                # piece A (chunks 0..PA-1 per core, processed first) and
                # piece B; RS of piece A overlaps piece B's aggregation.
                if is_final:
                    rs2_in = dram_rs.tile([N_CORES * OUT, SLOTS], F16)
                    rs2_out = dram_rs.tile([OUT, SLOTS], F16)
                    rsA_in = rsB_in = rsA_out = rsB_out = None
                else:
                    rsA_in = dram_rs.tile([N_CORES * P, PA * P], F16)
                    rsA_out = dram_rs.tile([P, PA * P], F16)
                    rsB_in = dram_rs.tile([N_CORES * P, PB * P], F16)
                    rsB_out = dram_rs.tile([P, PB * P], F16)
                    rs2_in = rs2_out = None

                st = {"agg": None, "z2": None}

                def do_flush(c0, b, sel, gath, is_final):
                    ps_a = psA.tile([P, BANKG * P], F32, space="PSUM")
                    for ci in range(BANKG):
                        c = b * BANKG + ci  # chunk within call
                        for t in range(KT):
                            rhs = bass.AP(
                                tensor=sel.tensor,
                                offset=sel[:].offset + (c * P * KT + t),
                                ap=[sel[:].ap[0], [KT, P]],
                            )
                            nc.tensor.matmul(
                                out=ps_a[:, ci * P : (ci + 1) * P],
                                lhsT=gath[:, c * KT + t, :],
                                rhs=rhs,
                                start=(t == 0),
                                stop=(t == KT - 1),
                            )
                    pp = c0 + b * BANKG  # processing position of flush start
                    in_a = pp < N_CORES * PA
                    if in_a:
                        gcore, gch = pp // PA, pp % PA
                        plen = PA
                    else:
                        q = pp - N_CORES * PA
                        gcore, gch = q // PB, q % PB
                        plen = PB
                    if st["agg"] is None:
                        st["agg"] = st_pool.tile(
                            [P, plen * P], F16,
                            tag="stA" if in_a else "stB",
                            name="aggst",
                        )
                        if is_final:
                            st["z2"] = z2_pool.tile(
                                [OUT, plen * P], F16,
                                tag="z2A" if in_a else "z2B",
                                name="z2st",
                            )
                    stage = st["agg"]
                    cs = slice(gch * P, gch * P + BANKG * P)
                    # flush: fp32 PSUM -> fp16 staging; alternate engines so
                    # neither ACT nor DVE falls behind the gather cadence
                    if b % 2 == 0:
                        nc.scalar.activation(
                            out=stage[:, cs], in_=ps_a[:],
                            func=mybir.ActivationFunctionType.Copy,
                        )
                    else:
                        nc.vector.tensor_copy(out=stage[:, cs], in_=ps_a[:])
                    if is_final:
                        # transform partials by Wrel2 before the reduce
                        ps_b = psZ.tile([OUT, BANKG * P], F32, space="PSUM", tag="psz")
                        nc.tensor.matmul(
                            out=ps_b[:], lhsT=w2_sb[:, :OUT], rhs=stage[:, cs],
                            start=True, stop=True,
                        )
                        nc.scalar.activation(
                            out=st["z2"][:, cs], in_=ps_b[:],
                            func=mybir.ActivationFunctionType.Copy,
                        )
                    if is_final:
                        if gch + BANKG == plen:  # core piece complete
                            coff = 0 if in_a else PA * P
                            dst = bass.AP(
                                tensor=rs2_in.tensor,
                                offset=rs2_in[:].offset
                                + gcore * OUT * SLOTS + coff,
                                ap=[[SLOTS, OUT], [1, plen * P]],
                            )
                            nc.sync.dma_start(out=dst, in_=st["z2"][:])
                            st["agg"] = st["z2"] = None
                    else:
                        # staged writes with a small final piece, so the
                        # core-piece's last write queues only a short
                        # transfer behind the saturated gather stream
                        marks = (18, 33, 36) if in_a else (12, 15)
                        if gch + BANKG in marks:
                            i = marks.index(gch + BANKG)
                            w0 = (marks[i - 1] if i else 0) * P
                            wend = (gch + BANKG) * P
                            rst = rsA_in if in_a else rsB_in
                            dst = bass.AP(
                                tensor=rst.tensor,
                                offset=rst[:].offset
                                + gcore * P * plen * P + w0,
                                ap=[[plen * P, P], [1, wend - w0]],
                            )
                            nc.sync.dma_start(out=dst, in_=stage[:, w0:wend])
                            if gch + BANKG == plen:
                                st["agg"] = st["z2"] = None

                def do_call(call, l, is_final):
                    c0 = call * CPG
                    gath = g_pool.tile([P, CPG * KT, D], F16, tag="gath")
                    src_tab = p_xloc[:] if l == 0 else h_tab[:]
                    nc.gpsimd.dma_gather(
                        out_ap=gath[:],
                        in_ap=src_tab,
                        idxs_ap=idx_sb[:, call * ICALL : (call + 1) * ICALL],
                        num_idxs=CPG * KT * P,
                        num_idxs_reg=nidx_reg,
                        elem_size=D,
                        single_packet=False,
                    )
                    # sel one-hot for the call's chunks: out [p, c, w, t]
                    sel = s_pool.tile([P, CPG, P, KT], F16)
                    dv = dstv_sb[:, c0 * KT : (c0 + CPG) * KT]
                    in0 = bass.AP(
                        tensor=dstv_sb.tensor,
                        offset=dv.offset,
                        ap=[dv.ap[0], [KT, CPG], [0, P], [1, KT]],
                    )
                    in1 = bass.AP(
                        tensor=iotar_sb.tensor,
                        offset=iotar_sb[:].offset,
                        ap=[iotar_sb[:].ap[0], [0, CPG], [KT, P], [1, KT]],
                    )
                    nc.vector.tensor_tensor(
                        out=sel[:], in0=in0, in1=in1, op=mybir.AluOpType.is_equal
                    )
                    # segment-sum matmuls, PSUM bank per BANKG chunks
                    for b in range(CPG // BANKG):
                        do_flush(c0, b, sel, gath, is_final)

                CALLS_A = N_CORES * PA // CPG
                for call in range(CALLS):
                    do_call(call, l, is_final)
                    if not is_final and call == CALLS_A + 2:
                        # piece-A reduce, dispatched a few calls into piece B
                        # so its sem waits don't head-of-line block gathers
                        nc.gpsimd.collective_compute(
                            "ReduceScatter", mybir.AluOpType.add,
                            replica_groups=rg,
                            ins=[rsA_in.opt()], outs=[rsA_out.opt()],
                        )

                if is_final:
                    nc.gpsimd.collective_compute(
                        "ReduceScatter", mybir.AluOpType.add, replica_groups=rg,
                        ins=[rs2_in.opt()], outs=[rs2_out.opt()],
                    )
                else:
                    nc.gpsimd.collective_compute(
                        "ReduceScatter", mybir.AluOpType.add, replica_groups=rg,
                        ins=[rsB_in.opt()], outs=[rsB_out.opt()],
                    )

                # ---------------- dense transform on own chunks ------------
                if is_final:
                    # z4 = rs2_out (agg @ Wrel2 summed) + Wroot2^T h + b2
                    agg2_sb = agg_pool.tile([OUT, SLOTS], F16, tag="agg")
                    nc.sync.dma_start(out=agg2_sb[:], in_=rs2_out[:])
                    for zb in range((CHUNKS + ZBLK - 1) // ZBLK):
                        w = min(ZBLK * P, SLOTS - zb * ZBLK * P)
                        cs = slice(zb * ZBLK * P, zb * ZBLK * P + w)
                        ps_z = psZ.tile([OUT, ZBLK * P], F32, space="PSUM", tag="psz")
                        nc.tensor.matmul(
                            out=ps_z[:, :w], lhsT=w2_sb[:, OUT : 2 * OUT],
                            rhs=hT_prev[:, cs], start=True, stop=True,
                        )
                        nc.vector.tensor_tensor(
                            out=out_sb[:, cs], in0=ps_z[:, :w],
                            in1=agg2_sb[:, cs], op=mybir.AluOpType.add,
                        )
                    nc.vector.tensor_scalar(
                        out=out_sb[:], in0=out_sb[:], scalar1=b2_sb[:],
                        scalar2=None, op0=mybir.AluOpType.add,
                    )
                    nc.sync.dma_start(out=p_out[:], in_=out_sb[:])
                    continue

                agg_sb = agg_pool.tile([P, SLOTS], F16, tag="agg")
                nc.sync.dma_start(out=agg_sb[:, : PA * P], in_=rsA_out[:])
                nc.sync.dma_start(out=agg_sb[:, PA * P :], in_=rsB_out[:])

                z_all = z_pool.tile([P, SLOTS], F16, tag="z")
                stats = stat_pool.tile([P, CHUNKS, nc.vector.BN_STATS_DIM], F32)
                w_rel = wrel_sb[:, l * D : (l + 1) * D]
                w_root = wroot_sb[:, l * D : (l + 1) * D]

                def do_zblock(zb):
                    w = min(ZBLK * P, SLOTS - zb * ZBLK * P)
                    cs = slice(zb * ZBLK * P, zb * ZBLK * P + w)
                    ps_z = psZ.tile([P, ZBLK * P], F32, space="PSUM", tag="psz")
                    nc.tensor.matmul(
                        out=ps_z[:, :w], lhsT=w_rel, rhs=agg_sb[:, cs],
                        start=True, stop=False,
                    )
                    nc.tensor.matmul(
                        out=ps_z[:, :w], lhsT=w_root, rhs=hT_prev[:, cs],
                        start=False, stop=True,
                    )
                    nc.scalar.activation(
                        out=z_all[:, cs], in_=ps_z[:, :w],
                        func=mybir.ActivationFunctionType.Copy,
                    )
                    for ci in range(w // P):
                        c = zb * ZBLK + ci
                        nc.vector.bn_stats(
                            out=stats[:, c, :],
                            in_=z_all[:, c * P : (c + 1) * P],
                        )

                for zb in range((CHUNKS + ZBLK - 1) // ZBLK):
                    do_zblock(zb)

                # ---------------- BatchNorm over all nodes -----------------
                bs = bn_pool.tile([P, 16], F32)
                mv = bs[:, 0:2]
                with tc.high_priority():
                    nc.vector.bn_aggr(out=mv, in_=stats[:])
                cc_sb = bs[:, 3:5]
                with tc.high_priority():
                    nc.vector.tensor_copy(out=cc_sb[:, 0:1], in_=mv[:, 0:1])
                    nc.vector.tensor_scalar(
                        out=cc_sb[:, 1:2], in0=mv[:, 0:1], scalar1=mv[:, 0:1],
                        scalar2=mv[:, 1:2], op0=mybir.AluOpType.mult,
                        op1=mybir.AluOpType.add,
                    )
                cc_in = dram_cc.tile([P, 2], F32)
                cc_out = dram_cc.tile([P * N_CORES, 2], F32, addr_space="Shared")
                nc.sync.dma_start(out=cc_in[:], in_=cc_sb)
                nc.gpsimd.collective_compute(
                    "AllGather", mybir.AluOpType.bypass, replica_groups=rg,
                    ins=[cc_in.opt()], outs=[cc_out.opt()],
                )
                cc_all = bn_pool.tile([P, 2, N_CORES], F32)
                cc_src = bass.AP(
                    tensor=cc_out.tensor,
                    offset=cc_out[:].offset,
                    ap=[[2, P], [1, 2], [2 * P, N_CORES]],
                )
                nc.sync.dma_start(out=cc_all[:], in_=cc_src)
                cc_res = bs[:, 5:7]
                nc.vector.tensor_reduce(
                    out=cc_res.rearrange("p (a b) -> p a b", a=2),
                    in_=cc_all[:],
                    axis=mybir.AxisListType.X,
                    op=mybir.AluOpType.add,
                )
                # mu = C/8 * sum(mean_c); E2 = C/8 * sum(E2_c); var = E2 - mu^2
                mu = bs[:, 7:8]
                nc.vector.tensor_scalar(
                    out=mu, in0=cc_res[:, 0:1], scalar2=None,
                    op0=mybir.AluOpType.mult, scalar1=STATC / N_CORES,
                )
                var = bs[:, 8:9]
                nc.vector.tensor_scalar(
                    out=var, in0=cc_res[:, 1:2], scalar2=None,
                    op0=mybir.AluOpType.mult, scalar1=STATC / N_CORES,
                )
                mu2 = bs[:, 9:10]
                nc.vector.tensor_tensor(
                    out=mu2, in0=mu, in1=mu, op=mybir.AluOpType.mult
                )
                nc.vector.tensor_tensor(
                    out=var, in0=var, in1=mu2, op=mybir.AluOpType.subtract
                )
                rstd = bs[:, 10:11]
                nc.scalar.activation(
                    out=rstd, in_=var,
                    func=mybir.ActivationFunctionType.Sqrt,
                    bias=eps_sb[:], scale=1.0,
                )
                nc.vector.reciprocal(out=rstd, in_=rstd)
                scale = bs[:, 11:12]
                nc.vector.tensor_tensor(
                    out=scale, in0=rstd, in1=gammaT_sb[:, l : l + 1],
                    op=mybir.AluOpType.mult,
                )
                shift = bs[:, 12:13]
                nc.vector.tensor_tensor(
                    out=shift, in0=mu, in1=scale, op=mybir.AluOpType.mult
                )
                nc.vector.tensor_tensor(
                    out=shift, in0=betaT_sb[:, l : l + 1], in1=shift,
                    op=mybir.AluOpType.subtract,
                )

                # ---------------- BN apply + rebuild local table -----------
                hT_new = hT_pool.tile([P, SLOTS], F16, tag="hT")
                h_tab = dram_tab.tile([SLOTS, D], F16)

                def do_table_block(cb):
                    gs = slice(cb * TBLK * P, (cb + 1) * TBLK * P)
                    nc.scalar.activation(
                        out=hT_new[:, gs], in_=z_all[:, gs],
                        func=mybir.ActivationFunctionType.Relu,
                        bias=shift, scale=scale,
                    )
                    # zero pad slots: next layer's Wroot term and BN stats
                    # need exact zeros there
                    nc.vector.tensor_tensor(
                        out=hT_new[:, gs], in0=hT_new[:, gs],
                        in1=mask_sb[:, gs], op=mybir.AluOpType.mult,
                    )
                    # xbar transpose to node-major, then one table write
                    tt = t16_pool.tile([P, TBLK, P], F16)
                    nc.scalar.dma_start(out=tt[:], in_=hT_new[:, gs],
                                        transpose=True)
                    dst = bass.AP(
                        tensor=h_tab.tensor,
                        offset=h_tab[:].offset + cb * TBLK * P * D,
                        ap=[[D, P], [P * D, TBLK], [1, D]],
                    )
                    nc.sync.dma_start(out=dst, in_=tt[:])

                for cb in range(CHUNKS // TBLK):
                    do_table_block(cb)
                hT_prev = hT_new

    lower_extended_insts(nc)
    _split_multiwait(nc)
    return nc


_PROGRAM_CACHE = {}


def _get_program():
    if "p" not in _PROGRAM_CACHE:
        _PROGRAM_CACHE["p"] = build_program()
    return _PROGRAM_CACHE["p"]


def _make_in_maps(idx_cores, dstv_cores, x_loc, xT_loc, mask,
                  Wrel, Wroot, b, gamma, beta, Wrel2, Wroot2, b2):
    iotar = np.repeat(np.arange(P, dtype=np.float16), KT)[None, :].repeat(P, 0)
    w2 = np.concatenate(
        [np.asarray(Wrel2, np.float32), np.asarray(Wroot2, np.float32)], axis=1
    )
    common = dict(
        wrel=np.ascontiguousarray(np.asarray(Wrel, np.float16)),
        wroot=np.ascontiguousarray(np.asarray(Wroot, np.float16)),
        w2=np.ascontiguousarray(w2.astype(np.float16)),
        gammaT=np.ascontiguousarray(np.asarray(gamma, np.float32).T),
        betaT=np.ascontiguousarray(np.asarray(beta, np.float32).T),
        b2=np.asarray(b2, np.float32).reshape(OUT, 1),
        iotar=np.ascontiguousarray(iotar),
    )
    in_maps = []
    for c in range(N_CORES):
        m = dict(common)
        m["x_loc"] = x_loc[c]
        m["xT_loc"] = xT_loc[c]
        m["gidx"] = idx_cores[c]
        m["dstv"] = dstv_cores[c]
        m["mask16"] = np.ascontiguousarray(mask[c])
        in_maps.append(m)
    return in_maps


def run(x, edge_index, Wrel, Wroot, b, gamma, beta, Wrel2, Wroot2, b2):
    """Returns (output [N, OUT] float32, nc) — nc exposed for profiling.

    Note: inner-layer GraphConv biases `b` are mathematically absorbed by
    training-mode BatchNorm and intentionally unused.
    """
    newid, idx_cores, dstv_cores, x_loc, xT_loc, mask = _preprocess(x, edge_index)
    nc = _get_program()
    in_maps = _make_in_maps(
        idx_cores, dstv_cores, x_loc, xT_loc, mask,
        Wrel, Wroot, b, gamma, beta, Wrel2, Wroot2, b2,
    )
    from concourse.bass_utils import run_bass_kernel_spmd

    res = run_bass_kernel_spmd(nc, in_maps, list(range(N_CORES)))
    full = np.concatenate(
        [res.results[c]["z4T"].T for c in range(N_CORES)], axis=0
    )  # [N_PAD, OUT]
    return full[newid].astype(np.float32), nc


def kernel(**inputs):
    out, _ = run(**{k: np.asarray(v) for k, v in inputs.items()})
    return out


# revision 3
# speedup vs baseline: 1.0031x; 1.0031x over previous
"""GNN message passing (3x GraphConv+BN+ReLU, final GraphConv) on 8 trn2 cores.

v2: source-sharded partial aggregation + ReduceScatter.
  - Nodes partitioned 8 cores x 51 chunks x 128 slots (N_PAD=52224), with a
    two-phase balance so every (src core, dst chunk) has <=256 edges ->
    uniform 2 tiles of 128 edges per chunk, minimal padding.
  - Each core keeps a LOCAL node-major fp16 table of its own nodes; per layer
    it gathers its edges' source rows (indirect DMA), segment-sums them into
    partial aggregates for ALL 408 dst chunks via one-hot matmuls in PSUM,
    writes fp16 partials to DRAM, and a ReduceScatter(add) delivers each
    core's own aggregated chunks (output 8x smaller than an AllGather).
  - GraphConv bias is absorbed by training-mode BatchNorm (shift-invariant)
    and dropped for inner layers; BN stats use a N_PAD/N correction with pad
    slots pinned to exact zero (mask folded into the transpose copy).
  - Final layer transforms partials by Wrel2 BEFORE the ReduceScatter, so the
    last collective output is only [2, 6528].
"""

import sys

import numpy as np

sys.path.insert(0, "/opt/trn_rl_repo")

import concourse.bass as bass  # noqa: E402
import concourse.mybir as mybir  # noqa: E402
import concourse.tile as tile  # noqa: E402
from concourse.vector_clock import ScopedClock  # noqa: E402
from concourse import library_config  # noqa: E402
from concourse.library_overlay import lower_extended_insts  # noqa: E402

N = 50000
E = 800000
D = 128
L = 3
OUT = 2
EPS = 1e-5
N_CORES = 8
P = 128
CHUNKS = 51                      # dst chunks per core
C_TOT = N_CORES * CHUNKS         # 408
SLOTS = CHUNKS * P               # 6528 per core
N_PAD = N_CORES * SLOTS          # 52224
KT = 2                           # edge tiles per chunk
TILES = C_TOT * KT               # 816 tiles per core per layer
NIDX = TILES * P                 # 104448 gathered rows per core per layer
CPG = 12                         # chunks per gather call (408 = 34*12)
CALLS = C_TOT // CPG             # 34
BANKG = 3                        # chunks per PSUM bank flush (51 = 17*3)
ZBLK = 4                         # own chunks per dense-z matmul block
TBLK = 17                        # chunks per table-rebuild block (51 = 3*17)
# RS pieces: (chunk offset within core, chunks, staged-write marks).
# Earlier pieces are processed (and reduced) first so their ReduceScatter
# overlaps the remaining aggregation.
PIECES = ((0, 36, (18, 33, 36)), (36, 15, (12, 15)))
_PSTART = tuple(pc0 * N_CORES for pc0, _, _ in PIECES)


def _piece_of(pp):
    """processing position -> (piece index, core, chunk-within-piece)"""
    for pi in range(len(PIECES) - 1, -1, -1):
        if pp >= _PSTART[pi]:
            q = pp - _PSTART[pi]
            plen = PIECES[pi][1]
            return pi, q // plen, q % plen
    raise AssertionError
STATC = 52224.0 / 50000.0        # pad-slot correction for BN stats

F16 = mybir.dt.float16
F32 = mybir.dt.float32

# ---------------------------------------------------------------------------
# walrus in this container accepts at most ONE semaphore wait per instruction.
# Patch the Tile exit drain and add a post-pass splitting multi-wait insts.
# ---------------------------------------------------------------------------
_MAX_WAITS = 1


def _drain_and_barrier(self, tick_clock, wait_clock):
    nc = self.nc
    drain_inst = nc.sync.drain()
    wait_clock.add_sem_waits(
        drain_inst.ins, ScopedClock({None: tick_clock.global_clock})
    )
    si = drain_inst.ins.sync_info
    if si is not None and si.on_wait is not None and len(si.on_wait) > _MAX_WAITS:
        waits = list(si.on_wait)
        si.on_wait = waits[:_MAX_WAITS]
        rest = waits[_MAX_WAITS:]
        for i in range(0, len(rest), _MAX_WAITS):
            nop = nc.sync.nop(nofuse=True)
            nop.ins.sync_info = mybir.SyncInfo(
                on_wait=rest[i : i + _MAX_WAITS], on_update=[]
            )
    nc.all_engine_barrier()
    assert self.sems is not None
    popped = nc._tile_sem_poison_stack.pop()
    assert popped is self._sem_poison
    nc.clear_and_free_semaphores(list(self.sems.allocated().values()))
    nc.all_engine_barrier()


tile.TileContext._drain_and_barrier = _drain_and_barrier


def _split_multiwait(nc):
    n_split = 0
    for fn in nc.m.functions:
        for blk in fn.blocks:
            out = []
            for inst in blk.instructions:
                si = inst.sync_info
                if si is not None and si.on_wait and len(si.on_wait) > _MAX_WAITS:
                    waits = list(si.on_wait)
                    si.on_wait = waits[-_MAX_WAITS:]
                    rest = waits[:-_MAX_WAITS]
                    for i in range(0, len(rest), _MAX_WAITS):
                        n_split += 1
                        out.append(
                            mybir.InstNoOp(
                                name=f"{inst.name}-ws{i}",
                                engine=inst.engine,
                                ins=[],
                                outs=[],
                                bass_nofuse=True,
                                sync_info=mybir.SyncInfo(
                                    on_wait=rest[i : i + _MAX_WAITS], on_update=[]
                                ),
                                debug=inst.debug,
                            )
                        )
                out.append(inst)
            blk.instructions[:] = out
    return n_split


# ---------------------------------------------------------------------------
# Host-side graph partitioning
# ---------------------------------------------------------------------------
def _partition(src, dst):
    """Two-phase partition: nodes -> cores (phase 1), then per core nodes ->
    51 chunks (phase 2) balancing the 8-dim in-edges-by-src-core vector with
    hard cap 256 per (src core, chunk)."""
    deg_out = np.bincount(src, minlength=N)
    deg_in = np.bincount(dst, minlength=N)

    order = np.argsort(-(deg_in + deg_out), kind="stable")
    core_of = np.empty(N, np.int32)
    tot = np.zeros(N_CORES)
    cnt = np.zeros(N_CORES, np.int64)
    cap_nodes = CHUNKS * P
    for nd in order:
        score = tot + 1e12 * (cnt >= cap_nodes)
        c = int(np.argmin(score))
        core_of[nd] = c
        tot[c] += deg_in[nd] + deg_out[nd]
        cnt[c] += 1

    vec = np.zeros((N, N_CORES), np.int32)
    np.add.at(vec, (dst, core_of[src]), 1)

    chunk_of = np.full(N, -1, np.int32)
    slot_of = np.full(N, -1, np.int32)
    cap_edge = KT * P
    max_cell = 0
    for k in range(N_CORES):
        nodes = np.where(core_of == k)[0]
        v = vec[nodes]
        o = np.argsort(-v.sum(1), kind="stable")
        fill = np.zeros((CHUNKS, N_CORES), np.int64)
        nfill = np.zeros(CHUNKS, np.int64)
        for i in o:
            w = v[i]
            res = fill + w[None, :]
            over = np.maximum(res - cap_edge, 0).sum(1)
            score = over * 100000 + res.max(1) * 100 + nfill
            score[nfill >= P] = 1 << 60
            g = int(np.argmin(score))
            nd = nodes[i]
            chunk_of[nd] = g
            slot_of[nd] = nfill[g]
            fill[g] += w
            nfill[g] += 1
        max_cell = max(max_cell, int(fill.max()))
    return core_of, chunk_of, slot_of, max_cell


def _preprocess(x, edge_index):
    x = np.asarray(x, np.float32)
    ei = np.asarray(edge_index)
    src = ei[0].astype(np.int64)
    dst = ei[1].astype(np.int64)

    core_of, chunk_of, slot_of, max_cell = _partition(src, dst)
    spill = max_cell > KT * P  # overflow fallback: caller bumps balancing
    assert not spill, f"partition overflow: max cell {max_cell}"

    newid = (
        core_of.astype(np.int64) * SLOTS
        + chunk_of.astype(np.int64) * P
        + slot_of
    )

    # Edge streams per owning (source) core, ordered by PROCESSING position:
    # RS piece 0 of all cores (core-major) first, then piece 1, then 2.
    g2proc = np.empty(C_TOT, np.int64)
    pos_i = 0
    for pc0, plen, _ in PIECES:
        for k in range(N_CORES):
            g2proc[k * CHUNKS + pc0 : k * CHUNKS + pc0 + plen] = np.arange(
                pos_i, pos_i + plen
            )
            pos_i += plen

    e_core = core_of[src]
    gchunk = g2proc[core_of[dst].astype(np.int64) * CHUNKS + chunk_of[dst]]
    loc_src = (newid[src] - e_core.astype(np.int64) * SLOTS).astype(np.int16)
    rel_dst = slot_of[dst].astype(np.int16)

    idx_cores = np.zeros((N_CORES, P, NIDX // 16), np.int16)
    dstv_cores = np.full((N_CORES, P, TILES), -1.0, np.float16)
    for c in range(N_CORES):
        m = e_core == c
        gc = gchunk[m]
        o = np.argsort(gc, kind="stable")
        gc = gc[o]
        ls = loc_src[m][o]
        rd = rel_dst[m][o]
        counts = np.bincount(gc, minlength=C_TOT)
        assert counts.max() <= KT * P
        starts = np.concatenate([[0], np.cumsum(counts)[:-1]])
        pos = np.arange(len(gc)) - starts[gc]
        flat = gc * (KT * P) + pos  # slot in the padded edge stream
        idx_full = np.zeros(NIDX, np.int16)
        idx_full[flat] = ls
        dv_full = np.full(NIDX, -1.0, np.float16)
        dv_full[flat] = rd.astype(np.float16)
        # gather idx wrap: logical i -> [i % 16, i // 16], replicated to 128
        wrapped = idx_full.reshape(NIDX // 16, 16).T  # [16, NIDX//16]
        idx_cores[c] = np.tile(wrapped, (8, 1))
        # dst values: edge slot (tile, p) -> dstv[p, tile]
        dstv_cores[c] = dv_full.reshape(TILES, P).T

    # local node tables
    x_pad = np.zeros((N_PAD, D), np.float32)
    x_pad[newid] = x
    x_loc = np.ascontiguousarray(
        x_pad.reshape(N_CORES, SLOTS, D).astype(np.float16)
    )
    xT_loc = np.ascontiguousarray(
        x_pad.reshape(N_CORES, SLOTS, D).transpose(0, 2, 1).astype(np.float16)
    )

    # pad-slot masks: [core, 1, SLOTS] fp16 rows, 1.0 for real nodes
    mask = np.zeros((N_CORES, 1, SLOTS), np.float16)
    sl = newid % SLOTS
    mask[core_of, 0, sl] = 1.0
    mask = np.broadcast_to(mask, (N_CORES, P, SLOTS))

    return newid, idx_cores, dstv_cores, x_loc, xT_loc, mask


# ---------------------------------------------------------------------------
# Device program
# ---------------------------------------------------------------------------
def build_program():
    nc = bass.Bass(num_devices=N_CORES)

    p_xloc = nc.declare_dram_parameter("x_loc", [SLOTS, D], F16, isOutput=False)
    p_xT = nc.declare_dram_parameter("xT_loc", [D, SLOTS], F16, isOutput=False)
    p_idx = nc.declare_dram_parameter("gidx", [P, NIDX // 16], mybir.dt.int16, isOutput=False)
    p_dstv = nc.declare_dram_parameter("dstv", [P, TILES], F16, isOutput=False)
    p_wrel = nc.declare_dram_parameter("wrel", [L, D, D], F16, isOutput=False)
    p_wroot = nc.declare_dram_parameter("wroot", [L, D, D], F16, isOutput=False)
    p_w2 = nc.declare_dram_parameter("w2", [D, 2 * OUT], F16, isOutput=False)
    p_gammaT = nc.declare_dram_parameter("gammaT", [D, L], F32, isOutput=False)
    p_betaT = nc.declare_dram_parameter("betaT", [D, L], F32, isOutput=False)
    p_b2 = nc.declare_dram_parameter("b2r", [1, OUT], F16, isOutput=False)
    p_iotar = nc.declare_dram_parameter("iotar", [P, P * KT], F16, isOutput=False)
    p_mask = nc.declare_dram_parameter("mask16", [P, SLOTS], F16, isOutput=False)
    p_out = nc.declare_dram_parameter("z4T", [OUT, SLOTS], F32, isOutput=True)

    rg = [list(range(N_CORES))]
    ICALL = CPG * KT * P // 16   # idx columns per gather call (384)

    with tile.TileContext(nc) as tc:
        with (
            tc.tile_pool(name="dram_tab", bufs=2, space="DRAM") as dram_tab,
            tc.tile_pool(name="dram_rs", bufs=2, space="DRAM") as dram_rs,
            tc.tile_pool(name="dram_cc", bufs=2, space="DRAM") as dram_cc,
            tc.tile_pool(name="singles", bufs=1) as singles,
            tc.tile_pool(name="hT", bufs=2) as hT_pool,
            tc.tile_pool(name="zb", bufs=1) as z_pool,
            tc.tile_pool(name="agg", bufs=1) as agg_pool,
            tc.tile_pool(name="gath", bufs=3) as g_pool,
            tc.tile_pool(name="sel", bufs=2) as s_pool,
            tc.tile_pool(name="aggst", bufs=3) as st_pool,
            tc.tile_pool(name="z2st", bufs=1) as z2_pool,
            tc.tile_pool(name="t16p", bufs=3) as t16_pool,
            tc.tile_pool(name="bns", bufs=2) as bn_pool,
            tc.tile_pool(name="stat", bufs=2) as stat_pool,
            tc.tile_pool(name="psA", bufs=4, space="PSUM") as psA,
            tc.tile_pool(name="psZ", bufs=2, space="PSUM") as psZ,
        ):
            with tc.high_priority():
                nc.gpsimd.load_library(library_config.mlp)
            nidx_reg = nc.gpsimd.to_reg(CPG * KT * P)

            # --- static data in SBUF ---
            idx_sb = singles.tile([P, NIDX // 16], mybir.dt.int16)
            nc.sync.dma_start(out=idx_sb[:], in_=p_idx[:])
            dstv_sb = singles.tile([P, TILES], F16)
            nc.sync.dma_start(out=dstv_sb[:], in_=p_dstv[:])
            iotar_sb = singles.tile([P, P * KT], F16)
            nc.sync.dma_start(out=iotar_sb[:], in_=p_iotar[:])
            mask_sb = singles.tile([P, SLOTS], F16)
            nc.sync.dma_start(out=mask_sb[:], in_=p_mask[:])
            wrel_sb = singles.tile([P, L * D], F16)
            wroot_sb = singles.tile([P, L * D], F16)
            for l in range(L):
                nc.sync.dma_start(out=wrel_sb[:, l * D : (l + 1) * D], in_=p_wrel[l])
                nc.sync.dma_start(out=wroot_sb[:, l * D : (l + 1) * D], in_=p_wroot[l])
            w2_sb = singles.tile([P, 2 * OUT], F16)
            nc.sync.dma_start(out=w2_sb[:], in_=p_w2[:])
            gammaT_sb = singles.tile([P, L], F32)
            nc.sync.dma_start(out=gammaT_sb[:], in_=p_gammaT[:])
            betaT_sb = singles.tile([P, L], F32)
            nc.sync.dma_start(out=betaT_sb[:], in_=p_betaT[:])
            b2_sb = singles.tile([1, OUT], F16)
            nc.sync.dma_start(out=b2_sb[:], in_=p_b2[:])
            ones_sb = singles.tile([1, ZBLK * P], F16)
            nc.vector.memset(ones_sb[:], 1.0)
            eps_sb = singles.tile([P, 1], F32)
            nc.vector.memset(eps_sb[:], EPS)

            hT_prev = hT_pool.tile([P, SLOTS], F16, tag="hT")
            nc.sync.dma_start(out=hT_prev[:], in_=p_xT[:])
            h_tab = None  # layer 0 gathers read p_xloc

            out_sb = singles.tile([OUT, SLOTS], F32)

            for l in range(L + 1):
                is_final = l == L

                # ------- partial aggregation over all 408 chunks, split into
                # piece A (chunks 0..PA-1 per core, processed first) and
                # piece B; RS of piece A overlaps piece B's aggregation.
                if is_final:
                    rs2_in = dram_rs.tile([N_CORES * OUT, SLOTS], F16)
                    rs2_out = dram_rs.tile([OUT, SLOTS], F16)
                    rs_ins = rs_outs = None
                else:
                    rs_ins, rs_outs = [], []
                    for pi, (pc0, plen, _) in enumerate(PIECES):
                        ri = dram_rs.tile(
                            [N_CORES * P, plen * P], F16, tag=f"rsi{pi}",
                            name="rs_in",
                        )
                        ro = dram_rs.tile(
                            [P, plen * P], F16, tag=f"rso{pi}", name="rs_out",
                        )
                        rs_ins.append(ri)
                        rs_outs.append(ro)
                    rs2_in = rs2_out = None

                st = {"agg": None, "z2": None}

                def do_flush(c0, b, sel, gath, is_final):
                    ps_a = psA.tile([P, BANKG * P], F32, space="PSUM")
                    for ci in range(BANKG):
                        c = b * BANKG + ci  # chunk within call
                        for t in range(KT):
                            rhs = bass.AP(
                                tensor=sel.tensor,
                                offset=sel[:].offset + (c * P * KT + t),
                                ap=[sel[:].ap[0], [KT, P]],
                            )
                            nc.tensor.matmul(
                                out=ps_a[:, ci * P : (ci + 1) * P],
                                lhsT=gath[:, c * KT + t, :],
                                rhs=rhs,
                                start=(t == 0),
                                stop=(t == KT - 1),
                            )
                    pp = c0 + b * BANKG  # processing position of flush start
                    pi, gcore, gch = _piece_of(pp)
                    pc0, plen, marks = PIECES[pi]
                    if st["agg"] is None:
                        st["agg"] = st_pool.tile(
                            [P, plen * P], F16, tag=f"st{pi}", name="aggst",
                        )
                        if is_final:
                            st["z2"] = z2_pool.tile(
                                [OUT, plen * P], F16, tag=f"z2p{pi}",
                                name="z2st",
                            )
                    stage = st["agg"]
                    cs = slice(gch * P, gch * P + BANKG * P)
                    # flush: fp32 PSUM -> fp16 staging; alternate engines so
                    # neither ACT nor DVE falls behind the gather cadence
                    if b % 2 == 0:
                        nc.scalar.activation(
                            out=stage[:, cs], in_=ps_a[:],
                            func=mybir.ActivationFunctionType.Copy,
                        )
                    else:
                        nc.vector.tensor_copy(out=stage[:, cs], in_=ps_a[:])
                    if is_final:
                        # transform partials by Wrel2 before the reduce
                        ps_b = psZ.tile([OUT, BANKG * P], F32, space="PSUM", tag="psz")
                        nc.tensor.matmul(
                            out=ps_b[:], lhsT=w2_sb[:, :OUT], rhs=stage[:, cs],
                            start=True, stop=True,
                        )
                        nc.scalar.activation(
                            out=st["z2"][:, cs], in_=ps_b[:],
                            func=mybir.ActivationFunctionType.Copy,
                        )
                        if gch + BANKG == plen:  # core piece complete
                            dst = bass.AP(
                                tensor=rs2_in.tensor,
                                offset=rs2_in[:].offset
                                + gcore * OUT * SLOTS + pc0 * P,
                                ap=[[SLOTS, OUT], [1, plen * P]],
                            )
                            nc.sync.dma_start(out=dst, in_=st["z2"][:])
                            st["agg"] = st["z2"] = None
                    else:
                        # staged writes with a small final piece, so the
                        # core-piece's last write queues only a short
                        # transfer behind the saturated gather stream
                        if gch + BANKG in marks:
                            i = marks.index(gch + BANKG)
                            w0 = (marks[i - 1] if i else 0) * P
                            wend = (gch + BANKG) * P
                            rst = rs_ins[pi]
                            dst = bass.AP(
                                tensor=rst.tensor,
                                offset=rst[:].offset
                                + gcore * P * plen * P + w0,
                                ap=[[plen * P, P], [1, wend - w0]],
                            )
                            nc.sync.dma_start(out=dst, in_=stage[:, w0:wend])
                            if gch + BANKG == plen:
                                st["agg"] = st["z2"] = None

                def do_call(call, l, is_final):
                    c0 = call * CPG
                    gath = g_pool.tile([P, CPG * KT, D], F16, tag="gath")
                    src_tab = p_xloc[:] if l == 0 else h_tab[:]
                    nc.gpsimd.dma_gather(
                        out_ap=gath[:],
                        in_ap=src_tab,
                        idxs_ap=idx_sb[:, call * ICALL : (call + 1) * ICALL],
                        num_idxs=CPG * KT * P,
                        num_idxs_reg=nidx_reg,
                        elem_size=D,
                        single_packet=False,
                    )
                    # sel one-hot for the call's chunks: out [p, c, w, t]
                    sel = s_pool.tile([P, CPG, P, KT], F16)
                    dv = dstv_sb[:, c0 * KT : (c0 + CPG) * KT]
                    in0 = bass.AP(
                        tensor=dstv_sb.tensor,
                        offset=dv.offset,
                        ap=[dv.ap[0], [KT, CPG], [0, P], [1, KT]],
                    )
                    in1 = bass.AP(
                        tensor=iotar_sb.tensor,
                        offset=iotar_sb[:].offset,
                        ap=[iotar_sb[:].ap[0], [0, CPG], [KT, P], [1, KT]],
                    )
                    nc.vector.tensor_tensor(
                        out=sel[:], in0=in0, in1=in1, op=mybir.AluOpType.is_equal
                    )
                    # segment-sum matmuls, PSUM bank per BANKG chunks
                    for b in range(CPG // BANKG):
                        do_flush(c0, b, sel, gath, is_final)

                # each piece's reduce dispatches two calls into the NEXT
                # piece so its sem waits don't head-of-line block gathers
                rs_at = {}
                acc = 0
                for pi in range(len(PIECES) - 1):
                    acc += PIECES[pi][1]
                    rs_at[N_CORES * acc // CPG + 1] = pi
                for call in range(CALLS):
                    do_call(call, l, is_final)
                    pi = rs_at.get(call)
                    if pi is not None and not is_final:
                        nc.gpsimd.collective_compute(
                            "ReduceScatter", mybir.AluOpType.add,
                            replica_groups=rg,
                            ins=[rs_ins[pi].opt()], outs=[rs_outs[pi].opt()],
                        )

                if is_final:
                    nc.gpsimd.collective_compute(
                        "ReduceScatter", mybir.AluOpType.add, replica_groups=rg,
                        ins=[rs2_in.opt()], outs=[rs2_out.opt()],
                    )
                else:
                    nc.gpsimd.collective_compute(
                        "ReduceScatter", mybir.AluOpType.add, replica_groups=rg,
                        ins=[rs_ins[-1].opt()], outs=[rs_outs[-1].opt()],
                    )

                # ---------------- dense transform on own chunks ------------
                if is_final:
                    # z4 = rs2_out (agg @ Wrel2 summed) + Wroot2^T h + b2
                    agg2_sb = agg_pool.tile([OUT, SLOTS], F16, tag="agg")
                    nc.sync.dma_start(out=agg2_sb[:], in_=rs2_out[:])
                    for zb in range((CHUNKS + ZBLK - 1) // ZBLK):
                        w = min(ZBLK * P, SLOTS - zb * ZBLK * P)
                        cs = slice(zb * ZBLK * P, zb * ZBLK * P + w)
                        ps_z = psZ.tile([OUT, ZBLK * P], F32, space="PSUM", tag="psz")
                        nc.tensor.matmul(
                            out=ps_z[:, :w], lhsT=w2_sb[:, OUT : 2 * OUT],
                            rhs=hT_prev[:, cs], start=True, stop=False,
                        )
                        nc.tensor.matmul(
                            out=ps_z[:, :w], lhsT=b2_sb[:],
                            rhs=ones_sb[:, :w], start=False, stop=True,
                        )
                        nc.vector.tensor_tensor(
                            out=out_sb[:, cs], in0=ps_z[:, :w],
                            in1=agg2_sb[:, cs], op=mybir.AluOpType.add,
                        )
                    nc.sync.dma_start(out=p_out[:], in_=out_sb[:])
                    continue

                agg_sb = agg_pool.tile([P, SLOTS], F16, tag="agg")
                for pi, (pc0, plen, _) in enumerate(PIECES):
                    nc.sync.dma_start(
                        out=agg_sb[:, pc0 * P : (pc0 + plen) * P],
                        in_=rs_outs[pi][:],
                    )

                z_all = z_pool.tile([P, SLOTS], F16, tag="z")
                stats = stat_pool.tile([P, CHUNKS, nc.vector.BN_STATS_DIM], F32)
                w_rel = wrel_sb[:, l * D : (l + 1) * D]
                w_root = wroot_sb[:, l * D : (l + 1) * D]

                def do_zblock(zb):
                    w = min(ZBLK * P, SLOTS - zb * ZBLK * P)
                    cs = slice(zb * ZBLK * P, zb * ZBLK * P + w)
                    ps_z = psZ.tile([P, ZBLK * P], F32, space="PSUM", tag="psz")
                    nc.tensor.matmul(
                        out=ps_z[:, :w], lhsT=w_rel, rhs=agg_sb[:, cs],
                        start=True, stop=False,
                    )
                    nc.tensor.matmul(
                        out=ps_z[:, :w], lhsT=w_root, rhs=hT_prev[:, cs],
                        start=False, stop=True,
                    )
                    nc.scalar.activation(
                        out=z_all[:, cs], in_=ps_z[:, :w],
                        func=mybir.ActivationFunctionType.Copy,
                    )
                    for ci in range(w // P):
                        c = zb * ZBLK + ci
                        nc.vector.bn_stats(
                            out=stats[:, c, :],
                            in_=z_all[:, c * P : (c + 1) * P],
                        )

                for zb in range((CHUNKS + ZBLK - 1) // ZBLK):
                    do_zblock(zb)

                # ---------------- BatchNorm over all nodes -----------------
                bs = bn_pool.tile([P, 16], F32)
                mv = bs[:, 0:2]
                with tc.high_priority():
                    nc.vector.bn_aggr(out=mv, in_=stats[:])
                cc_sb = bs[:, 3:5]
                with tc.high_priority():
                    nc.vector.tensor_copy(out=cc_sb[:, 0:1], in_=mv[:, 0:1])
                    nc.vector.tensor_scalar(
                        out=cc_sb[:, 1:2], in0=mv[:, 0:1], scalar1=mv[:, 0:1],
                        scalar2=mv[:, 1:2], op0=mybir.AluOpType.mult,
                        op1=mybir.AluOpType.add,
                    )
                cc_in = dram_cc.tile([P, 2], F32)
                cc_out = dram_cc.tile([P * N_CORES, 2], F32, addr_space="Shared")
                nc.sync.dma_start(out=cc_in[:], in_=cc_sb)
                nc.gpsimd.collective_compute(
                    "AllGather", mybir.AluOpType.bypass, replica_groups=rg,
                    ins=[cc_in.opt()], outs=[cc_out.opt()],
                )
                cc_all = bn_pool.tile([P, 2, N_CORES], F32)
                cc_src = bass.AP(
                    tensor=cc_out.tensor,
                    offset=cc_out[:].offset,
                    ap=[[2, P], [1, 2], [2 * P, N_CORES]],
                )
                nc.sync.dma_start(out=cc_all[:], in_=cc_src)
                cc_res = bs[:, 5:7]
                nc.vector.tensor_reduce(
                    out=cc_res.rearrange("p (a b) -> p a b", a=2),
                    in_=cc_all[:],
                    axis=mybir.AxisListType.X,
                    op=mybir.AluOpType.add,
                )
                # mu = C/8 * sum(mean_c); E2 = C/8 * sum(E2_c); var = E2 - mu^2
                mu = bs[:, 7:8]
                nc.vector.tensor_scalar(
                    out=mu, in0=cc_res[:, 0:1], scalar2=None,
                    op0=mybir.AluOpType.mult, scalar1=STATC / N_CORES,
                )
                var = bs[:, 8:9]
                nc.vector.tensor_scalar(
                    out=var, in0=cc_res[:, 1:2], scalar2=None,
                    op0=mybir.AluOpType.mult, scalar1=STATC / N_CORES,
                )
                mu2 = bs[:, 9:10]
                nc.vector.tensor_tensor(
                    out=mu2, in0=mu, in1=mu, op=mybir.AluOpType.mult
                )
                nc.vector.tensor_tensor(
                    out=var, in0=var, in1=mu2, op=mybir.AluOpType.subtract
                )
                rstd = bs[:, 10:11]
                nc.scalar.activation(
                    out=rstd, in_=var,
                    func=mybir.ActivationFunctionType.Sqrt,
                    bias=eps_sb[:], scale=1.0,
                )
                nc.vector.reciprocal(out=rstd, in_=rstd)
                scale = bs[:, 11:12]
                nc.vector.tensor_tensor(
                    out=scale, in0=rstd, in1=gammaT_sb[:, l : l + 1],
                    op=mybir.AluOpType.mult,
                )
                shift = bs[:, 12:13]
                nc.vector.tensor_tensor(
                    out=shift, in0=mu, in1=scale, op=mybir.AluOpType.mult
                )
                nc.vector.tensor_tensor(
                    out=shift, in0=betaT_sb[:, l : l + 1], in1=shift,
                    op=mybir.AluOpType.subtract,
                )

                # ---------------- BN apply + rebuild local table -----------
                hT_new = hT_pool.tile([P, SLOTS], F16, tag="hT")
                h_tab = dram_tab.tile([SLOTS, D], F16)

                def do_table_block(cb):
                    gs = slice(cb * TBLK * P, (cb + 1) * TBLK * P)
                    nc.scalar.activation(
                        out=hT_new[:, gs], in_=z_all[:, gs],
                        func=mybir.ActivationFunctionType.Relu,
                        bias=shift, scale=scale,
                    )
                    # zero pad slots: next layer's Wroot term and BN stats
                    # need exact zeros there
                    nc.vector.tensor_tensor(
                        out=hT_new[:, gs], in0=hT_new[:, gs],
                        in1=mask_sb[:, gs], op=mybir.AluOpType.mult,
                    )
                    # xbar transpose to node-major, then one table write
                    tt = t16_pool.tile([P, TBLK, P], F16)
                    nc.scalar.dma_start(out=tt[:], in_=hT_new[:, gs],
                                        transpose=True)
                    dst = bass.AP(
                        tensor=h_tab.tensor,
                        offset=h_tab[:].offset + cb * TBLK * P * D,
                        ap=[[D, P], [P * D, TBLK], [1, D]],
                    )
                    nc.sync.dma_start(out=dst, in_=tt[:])

                for cb in range(CHUNKS // TBLK):
                    do_table_block(cb)
                hT_prev = hT_new

    lower_extended_insts(nc)
    _split_multiwait(nc)
    return nc


_PROGRAM_CACHE = {}


def _get_program():
    if "p" not in _PROGRAM_CACHE:
        _PROGRAM_CACHE["p"] = build_program()
    return _PROGRAM_CACHE["p"]


def _make_in_maps(idx_cores, dstv_cores, x_loc, xT_loc, mask,
                  Wrel, Wroot, b, gamma, beta, Wrel2, Wroot2, b2):
    iotar = np.repeat(np.arange(P, dtype=np.float16), KT)[None, :].repeat(P, 0)
    w2 = np.concatenate(
        [np.asarray(Wrel2, np.float32), np.asarray(Wroot2, np.float32)], axis=1
    )
    common = dict(
        wrel=np.ascontiguousarray(np.asarray(Wrel, np.float16)),
        wroot=np.ascontiguousarray(np.asarray(Wroot, np.float16)),
        w2=np.ascontiguousarray(w2.astype(np.float16)),
        gammaT=np.ascontiguousarray(np.asarray(gamma, np.float32).T),
        betaT=np.ascontiguousarray(np.asarray(beta, np.float32).T),
        b2r=np.asarray(b2, np.float16).reshape(1, OUT),
        iotar=np.ascontiguousarray(iotar),
    )
    in_maps = []
    for c in range(N_CORES):
        m = dict(common)
        m["x_loc"] = x_loc[c]
        m["xT_loc"] = xT_loc[c]
        m["gidx"] = idx_cores[c]
        m["dstv"] = dstv_cores[c]
        m["mask16"] = np.ascontiguousarray(mask[c])
        in_maps.append(m)
    return in_maps


def run(x, edge_index, Wrel, Wroot, b, gamma, beta, Wrel2, Wroot2, b2):
    """Returns (output [N, OUT] float32, nc) — nc exposed for profiling.

    Note: inner-layer GraphConv biases `b` are mathematically absorbed by
    training-mode BatchNorm and intentionally unused.
    """
    newid, idx_cores, dstv_cores, x_loc, xT_loc, mask = _preprocess(x, edge_index)
    nc = _get_program()
    in_maps = _make_in_maps(
        idx_cores, dstv_cores, x_loc, xT_loc, mask,
        Wrel, Wroot, b, gamma, beta, Wrel2, Wroot2, b2,
    )
    from concourse.bass_utils import run_bass_kernel_spmd

    res = run_bass_kernel_spmd(nc, in_maps, list(range(N_CORES)))
    full = np.concatenate(
        [res.results[c]["z4T"].T for c in range(N_CORES)], axis=0
    )  # [N_PAD, OUT]
    return full[newid].astype(np.float32), nc


def kernel(**inputs):
    out, _ = run(**{k: np.asarray(v) for k, v in inputs.items()})
    return out


# revision 4
# speedup vs baseline: 1.0033x; 1.0002x over previous
"""GNN message passing (3x GraphConv+BN+ReLU, final GraphConv) on 8 trn2 cores.

v2: source-sharded partial aggregation + ReduceScatter.
  - Nodes partitioned 8 cores x 51 chunks x 128 slots (N_PAD=52224), with a
    two-phase balance so every (src core, dst chunk) has <=256 edges ->
    uniform 2 tiles of 128 edges per chunk, minimal padding.
  - Each core keeps a LOCAL node-major fp16 table of its own nodes; per layer
    it gathers its edges' source rows (indirect DMA), segment-sums them into
    partial aggregates for ALL 408 dst chunks via one-hot matmuls in PSUM,
    writes fp16 partials to DRAM, and a ReduceScatter(add) delivers each
    core's own aggregated chunks (output 8x smaller than an AllGather).
  - GraphConv bias is absorbed by training-mode BatchNorm (shift-invariant)
    and dropped for inner layers; BN stats use a N_PAD/N correction with pad
    slots pinned to exact zero (mask folded into the transpose copy).
  - Final layer transforms partials by Wrel2 BEFORE the ReduceScatter, so the
    last collective output is only [2, 6528].
"""

import sys

import numpy as np

sys.path.insert(0, "/opt/trn_rl_repo")

import concourse.bass as bass  # noqa: E402
import concourse.mybir as mybir  # noqa: E402
import concourse.tile as tile  # noqa: E402
from concourse.vector_clock import ScopedClock  # noqa: E402
from concourse import library_config  # noqa: E402
from concourse.library_overlay import lower_extended_insts  # noqa: E402

N = 50000
E = 800000
D = 128
L = 3
OUT = 2
EPS = 1e-5
N_CORES = 8
P = 128
CHUNKS = 51                      # dst chunks per core
C_TOT = N_CORES * CHUNKS         # 408
SLOTS = CHUNKS * P               # 6528 per core
N_PAD = N_CORES * SLOTS          # 52224
KT = 2                           # edge tiles per chunk
TILES = C_TOT * KT               # 816 tiles per core per layer
NIDX = TILES * P                 # 104448 gathered rows per core per layer
CPG = 12                         # chunks per gather call (408 = 34*12)
CALLS = C_TOT // CPG             # 34
BANKG = 3                        # chunks per PSUM bank flush (51 = 17*3)
ZBLK = 4                         # own chunks per dense-z matmul block
TBLK = 17                        # chunks per table-rebuild block (51 = 3*17)
# RS pieces: (chunk offset within core, chunks, staged-write marks).
# Earlier pieces are processed (and reduced) first so their ReduceScatter
# overlaps the remaining aggregation.
PIECES = ((0, 36, (18, 33, 36)), (36, 15, (12, 15)))
_PSTART = tuple(pc0 * N_CORES for pc0, _, _ in PIECES)


def _piece_of(pp):
    """processing position -> (piece index, core, chunk-within-piece)"""
    for pi in range(len(PIECES) - 1, -1, -1):
        if pp >= _PSTART[pi]:
            q = pp - _PSTART[pi]
            plen = PIECES[pi][1]
            return pi, q // plen, q % plen
    raise AssertionError
STATC = 52224.0 / 50000.0        # pad-slot correction for BN stats

F16 = mybir.dt.float16
F32 = mybir.dt.float32

# ---------------------------------------------------------------------------
# walrus in this container accepts at most ONE semaphore wait per instruction.
# Patch the Tile exit drain and add a post-pass splitting multi-wait insts.
# ---------------------------------------------------------------------------
_MAX_WAITS = 1


def _drain_and_barrier(self, tick_clock, wait_clock):
    nc = self.nc
    drain_inst = nc.sync.drain()
    wait_clock.add_sem_waits(
        drain_inst.ins, ScopedClock({None: tick_clock.global_clock})
    )
    si = drain_inst.ins.sync_info
    if si is not None and si.on_wait is not None and len(si.on_wait) > _MAX_WAITS:
        waits = list(si.on_wait)
        si.on_wait = waits[:_MAX_WAITS]
        rest = waits[_MAX_WAITS:]
        for i in range(0, len(rest), _MAX_WAITS):
            nop = nc.sync.nop(nofuse=True)
            nop.ins.sync_info = mybir.SyncInfo(
                on_wait=rest[i : i + _MAX_WAITS], on_update=[]
            )
    nc.all_engine_barrier()
    assert self.sems is not None
    popped = nc._tile_sem_poison_stack.pop()
    assert popped is self._sem_poison
    nc.clear_and_free_semaphores(list(self.sems.allocated().values()))
    nc.all_engine_barrier()


tile.TileContext._drain_and_barrier = _drain_and_barrier


def _split_multiwait(nc):
    n_split = 0
    for fn in nc.m.functions:
        for blk in fn.blocks:
            out = []
            for inst in blk.instructions:
                si = inst.sync_info
                if si is not None and si.on_wait and len(si.on_wait) > _MAX_WAITS:
                    waits = list(si.on_wait)
                    si.on_wait = waits[-_MAX_WAITS:]
                    rest = waits[:-_MAX_WAITS]
                    for i in range(0, len(rest), _MAX_WAITS):
                        n_split += 1
                        out.append(
                            mybir.InstNoOp(
                                name=f"{inst.name}-ws{i}",
                                engine=inst.engine,
                                ins=[],
                                outs=[],
                                bass_nofuse=True,
                                sync_info=mybir.SyncInfo(
                                    on_wait=rest[i : i + _MAX_WAITS], on_update=[]
                                ),
                                debug=inst.debug,
                            )
                        )
                out.append(inst)
            blk.instructions[:] = out
    return n_split


# ---------------------------------------------------------------------------
# Host-side graph partitioning
# ---------------------------------------------------------------------------
def _partition(src, dst):
    """Two-phase partition: nodes -> cores (phase 1), then per core nodes ->
    51 chunks (phase 2) balancing the 8-dim in-edges-by-src-core vector with
    hard cap 256 per (src core, chunk)."""
    deg_out = np.bincount(src, minlength=N)
    deg_in = np.bincount(dst, minlength=N)

    order = np.argsort(-(deg_in + deg_out), kind="stable")
    core_of = np.empty(N, np.int32)
    tot = np.zeros(N_CORES)
    cnt = np.zeros(N_CORES, np.int64)
    cap_nodes = CHUNKS * P
    for nd in order:
        score = tot + 1e12 * (cnt >= cap_nodes)
        c = int(np.argmin(score))
        core_of[nd] = c
        tot[c] += deg_in[nd] + deg_out[nd]
        cnt[c] += 1

    vec = np.zeros((N, N_CORES), np.int32)
    np.add.at(vec, (dst, core_of[src]), 1)

    chunk_of = np.full(N, -1, np.int32)
    slot_of = np.full(N, -1, np.int32)
    cap_edge = KT * P
    max_cell = 0
    for k in range(N_CORES):
        nodes = np.where(core_of == k)[0]
        v = vec[nodes]
        o = np.argsort(-v.sum(1), kind="stable")
        fill = np.zeros((CHUNKS, N_CORES), np.int64)
        nfill = np.zeros(CHUNKS, np.int64)
        for i in o:
            w = v[i]
            res = fill + w[None, :]
            over = np.maximum(res - cap_edge, 0).sum(1)
            score = over * 100000 + res.max(1) * 100 + nfill
            score[nfill >= P] = 1 << 60
            g = int(np.argmin(score))
            nd = nodes[i]
            chunk_of[nd] = g
            slot_of[nd] = nfill[g]
            fill[g] += w
            nfill[g] += 1
        max_cell = max(max_cell, int(fill.max()))
    return core_of, chunk_of, slot_of, max_cell


def _preprocess(x, edge_index):
    x = np.asarray(x, np.float32)
    ei = np.asarray(edge_index)
    src = ei[0].astype(np.int64)
    dst = ei[1].astype(np.int64)

    core_of, chunk_of, slot_of, max_cell = _partition(src, dst)
    spill = max_cell > KT * P  # overflow fallback: caller bumps balancing
    assert not spill, f"partition overflow: max cell {max_cell}"

    newid = (
        core_of.astype(np.int64) * SLOTS
        + chunk_of.astype(np.int64) * P
        + slot_of
    )

    # Edge streams per owning (source) core, ordered by PROCESSING position:
    # RS piece 0 of all cores (core-major) first, then piece 1, then 2.
    g2proc = np.empty(C_TOT, np.int64)
    pos_i = 0
    for pc0, plen, _ in PIECES:
        for k in range(N_CORES):
            g2proc[k * CHUNKS + pc0 : k * CHUNKS + pc0 + plen] = np.arange(
                pos_i, pos_i + plen
            )
            pos_i += plen

    e_core = core_of[src]
    gchunk = g2proc[core_of[dst].astype(np.int64) * CHUNKS + chunk_of[dst]]
    loc_src = (newid[src] - e_core.astype(np.int64) * SLOTS).astype(np.int16)
    rel_dst = slot_of[dst].astype(np.int16)

    idx_cores = np.zeros((N_CORES, P, NIDX // 16), np.int16)
    dstv_cores = np.full((N_CORES, P, TILES), -1.0, np.float16)
    for c in range(N_CORES):
        m = e_core == c
        gc = gchunk[m]
        o = np.argsort(gc, kind="stable")
        gc = gc[o]
        ls = loc_src[m][o]
        rd = rel_dst[m][o]
        counts = np.bincount(gc, minlength=C_TOT)
        assert counts.max() <= KT * P
        starts = np.concatenate([[0], np.cumsum(counts)[:-1]])
        pos = np.arange(len(gc)) - starts[gc]
        flat = gc * (KT * P) + pos  # slot in the padded edge stream
        idx_full = np.zeros(NIDX, np.int16)
        idx_full[flat] = ls
        dv_full = np.full(NIDX, -1.0, np.float16)
        dv_full[flat] = rd.astype(np.float16)
        # gather idx wrap: logical i -> [i % 16, i // 16], replicated to 128
        wrapped = idx_full.reshape(NIDX // 16, 16).T  # [16, NIDX//16]
        idx_cores[c] = np.tile(wrapped, (8, 1))
        # dst values: edge slot (tile, p) -> dstv[p, tile]
        dstv_cores[c] = dv_full.reshape(TILES, P).T

    # local node tables
    x_pad = np.zeros((N_PAD, D), np.float32)
    x_pad[newid] = x
    x_loc = np.ascontiguousarray(
        x_pad.reshape(N_CORES, SLOTS, D).astype(np.float16)
    )
    xT_loc = np.ascontiguousarray(
        x_pad.reshape(N_CORES, SLOTS, D).transpose(0, 2, 1).astype(np.float16)
    )

    # pad-slot masks: [core, 1, SLOTS] fp16 rows, 1.0 for real nodes
    mask = np.zeros((N_CORES, 1, SLOTS), np.float16)
    sl = newid % SLOTS
    mask[core_of, 0, sl] = 1.0
    mask = np.broadcast_to(mask, (N_CORES, P, SLOTS))

    return newid, idx_cores, dstv_cores, x_loc, xT_loc, mask


# ---------------------------------------------------------------------------
# Device program
# ---------------------------------------------------------------------------
def build_program():
    nc = bass.Bass(num_devices=N_CORES)

    p_xloc = nc.declare_dram_parameter("x_loc", [SLOTS, D], F16, isOutput=False)
    p_xT = nc.declare_dram_parameter("xT_loc", [D, SLOTS], F16, isOutput=False)
    p_idx = nc.declare_dram_parameter("gidx", [P, NIDX // 16], mybir.dt.int16, isOutput=False)
    p_dstv = nc.declare_dram_parameter("dstv", [P, TILES], F16, isOutput=False)
    p_wrel = nc.declare_dram_parameter("wrel", [L, D, D], F16, isOutput=False)
    p_wroot = nc.declare_dram_parameter("wroot", [L, D, D], F16, isOutput=False)
    p_w2 = nc.declare_dram_parameter("w2", [D, 2 * OUT], F16, isOutput=False)
    p_gammaT = nc.declare_dram_parameter("gammaT", [D, L], F32, isOutput=False)
    p_betaT = nc.declare_dram_parameter("betaT", [D, L], F32, isOutput=False)
    p_b2 = nc.declare_dram_parameter("b2r", [1, OUT], F16, isOutput=False)
    p_iotar = nc.declare_dram_parameter("iotar", [P, P * KT], F16, isOutput=False)
    p_mask = nc.declare_dram_parameter("mask16", [P, SLOTS], F16, isOutput=False)
    p_out = nc.declare_dram_parameter("z4T", [OUT, SLOTS], F32, isOutput=True)

    rg = [list(range(N_CORES))]
    ICALL = CPG * KT * P // 16   # idx columns per gather call (384)

    with tile.TileContext(nc) as tc:
        with (
            tc.tile_pool(name="dram_tab", bufs=2, space="DRAM") as dram_tab,
            tc.tile_pool(name="dram_rs", bufs=2, space="DRAM") as dram_rs,
            tc.tile_pool(name="dram_cc", bufs=2, space="DRAM") as dram_cc,
            tc.tile_pool(name="singles", bufs=1) as singles,
            tc.tile_pool(name="hT", bufs=2) as hT_pool,
            tc.tile_pool(name="zb", bufs=1) as z_pool,
            tc.tile_pool(name="agg", bufs=1) as agg_pool,
            tc.tile_pool(name="gath", bufs=3) as g_pool,
            tc.tile_pool(name="sel", bufs=2) as s_pool,
            tc.tile_pool(name="aggst", bufs=3) as st_pool,
            tc.tile_pool(name="z2st", bufs=1) as z2_pool,
            tc.tile_pool(name="t16p", bufs=3) as t16_pool,
            tc.tile_pool(name="bns", bufs=2) as bn_pool,
            tc.tile_pool(name="stat", bufs=2) as stat_pool,
            tc.tile_pool(name="psA", bufs=5, space="PSUM") as psA,
            tc.tile_pool(name="psZ", bufs=2, space="PSUM") as psZ,
        ):
            with tc.high_priority():
                nc.gpsimd.load_library(library_config.mlp)
            nidx_reg = nc.gpsimd.to_reg(CPG * KT * P)

            # --- static data in SBUF ---
            idx_sb = singles.tile([P, NIDX // 16], mybir.dt.int16)
            nc.sync.dma_start(out=idx_sb[:], in_=p_idx[:])
            dstv_sb = singles.tile([P, TILES], F16)
            nc.sync.dma_start(out=dstv_sb[:], in_=p_dstv[:])
            iotar_sb = singles.tile([P, P * KT], F16)
            nc.sync.dma_start(out=iotar_sb[:], in_=p_iotar[:])
            mask_sb = singles.tile([P, SLOTS], F16)
            nc.sync.dma_start(out=mask_sb[:], in_=p_mask[:])
            wrel_sb = singles.tile([P, L * D], F16)
            wroot_sb = singles.tile([P, L * D], F16)
            for l in range(L):
                nc.sync.dma_start(out=wrel_sb[:, l * D : (l + 1) * D], in_=p_wrel[l])
                nc.sync.dma_start(out=wroot_sb[:, l * D : (l + 1) * D], in_=p_wroot[l])
            w2_sb = singles.tile([P, 2 * OUT], F16)
            nc.sync.dma_start(out=w2_sb[:], in_=p_w2[:])
            gammaT_sb = singles.tile([P, L], F32)
            nc.sync.dma_start(out=gammaT_sb[:], in_=p_gammaT[:])
            betaT_sb = singles.tile([P, L], F32)
            nc.sync.dma_start(out=betaT_sb[:], in_=p_betaT[:])
            b2_sb = singles.tile([1, OUT], F16)
            nc.sync.dma_start(out=b2_sb[:], in_=p_b2[:])
            ones_sb = singles.tile([1, ZBLK * P], F16)
            nc.vector.memset(ones_sb[:], 1.0)
            eps_sb = singles.tile([P, 1], F32)
            nc.vector.memset(eps_sb[:], EPS)

            hT_prev = hT_pool.tile([P, SLOTS], F16, tag="hT")
            nc.sync.dma_start(out=hT_prev[:], in_=p_xT[:])
            h_tab = None  # layer 0 gathers read p_xloc

            out_sb = singles.tile([OUT, SLOTS], F32)

            for l in range(L + 1):
                is_final = l == L

                # ------- partial aggregation over all 408 chunks, split into
                # piece A (chunks 0..PA-1 per core, processed first) and
                # piece B; RS of piece A overlaps piece B's aggregation.
                if is_final:
                    rs2_in = dram_rs.tile([N_CORES * OUT, SLOTS], F16)
                    rs2_out = dram_rs.tile([OUT, SLOTS], F16)
                    rs_ins = rs_outs = None
                else:
                    rs_ins, rs_outs = [], []
                    for pi, (pc0, plen, _) in enumerate(PIECES):
                        ri = dram_rs.tile(
                            [N_CORES * P, plen * P], F16, tag=f"rsi{pi}",
                            name="rs_in",
                        )
                        ro = dram_rs.tile(
                            [P, plen * P], F16, tag=f"rso{pi}", name="rs_out",
                        )
                        rs_ins.append(ri)
                        rs_outs.append(ro)
                    rs2_in = rs2_out = None

                st = {"agg": None, "z2": None}

                def do_flush(c0, b, sel, gath, is_final):
                    ps_a = psA.tile([P, BANKG * P], F32, space="PSUM")
                    for ci in range(BANKG):
                        c = b * BANKG + ci  # chunk within call
                        for t in range(KT):
                            rhs = bass.AP(
                                tensor=sel.tensor,
                                offset=sel[:].offset + (c * P * KT + t),
                                ap=[sel[:].ap[0], [KT, P]],
                            )
                            nc.tensor.matmul(
                                out=ps_a[:, ci * P : (ci + 1) * P],
                                lhsT=gath[:, c * KT + t, :],
                                rhs=rhs,
                                start=(t == 0),
                                stop=(t == KT - 1),
                            )
                    pp = c0 + b * BANKG  # processing position of flush start
                    pi, gcore, gch = _piece_of(pp)
                    pc0, plen, marks = PIECES[pi]
                    if st["agg"] is None:
                        st["agg"] = st_pool.tile(
                            [P, plen * P], F16, tag=f"st{pi}", name="aggst",
                        )
                        if is_final:
                            st["z2"] = z2_pool.tile(
                                [OUT, plen * P], F16, tag=f"z2p{pi}",
                                name="z2st",
                            )
                    stage = st["agg"]
                    cs = slice(gch * P, gch * P + BANKG * P)
                    # flush: fp32 PSUM -> fp16 staging; alternate engines so
                    # neither ACT nor DVE falls behind the gather cadence
                    if b % 2 == 0:
                        nc.scalar.activation(
                            out=stage[:, cs], in_=ps_a[:],
                            func=mybir.ActivationFunctionType.Copy,
                        )
                    else:
                        nc.vector.tensor_copy(out=stage[:, cs], in_=ps_a[:])
                    if is_final:
                        # transform partials by Wrel2 before the reduce
                        ps_b = psZ.tile([OUT, BANKG * P], F32, space="PSUM", tag="psz")
                        nc.tensor.matmul(
                            out=ps_b[:], lhsT=w2_sb[:, :OUT], rhs=stage[:, cs],
                            start=True, stop=True,
                        )
                        nc.scalar.activation(
                            out=st["z2"][:, cs], in_=ps_b[:],
                            func=mybir.ActivationFunctionType.Copy,
                        )
                        if gch + BANKG == plen:  # core piece complete
                            dst = bass.AP(
                                tensor=rs2_in.tensor,
                                offset=rs2_in[:].offset
                                + gcore * OUT * SLOTS + pc0 * P,
                                ap=[[SLOTS, OUT], [1, plen * P]],
                            )
                            nc.sync.dma_start(out=dst, in_=st["z2"][:])
                            st["agg"] = st["z2"] = None
                    else:
                        # staged writes with a small final piece, so the
                        # core-piece's last write queues only a short
                        # transfer behind the saturated gather stream
                        if gch + BANKG in marks:
                            i = marks.index(gch + BANKG)
                            w0 = (marks[i - 1] if i else 0) * P
                            wend = (gch + BANKG) * P
                            rst = rs_ins[pi]
                            dst = bass.AP(
                                tensor=rst.tensor,
                                offset=rst[:].offset
                                + gcore * P * plen * P + w0,
                                ap=[[plen * P, P], [1, wend - w0]],
                            )
                            nc.sync.dma_start(out=dst, in_=stage[:, w0:wend])
                            if gch + BANKG == plen:
                                st["agg"] = st["z2"] = None

                def do_call(call, l, is_final):
                    c0 = call * CPG
                    gath = g_pool.tile([P, CPG * KT, D], F16, tag="gath")
                    src_tab = p_xloc[:] if l == 0 else h_tab[:]
                    nc.gpsimd.dma_gather(
                        out_ap=gath[:],
                        in_ap=src_tab,
                        idxs_ap=idx_sb[:, call * ICALL : (call + 1) * ICALL],
                        num_idxs=CPG * KT * P,
                        num_idxs_reg=nidx_reg,
                        elem_size=D,
                        single_packet=False,
                    )
                    # sel one-hot for the call's chunks: out [p, c, w, t]
                    sel = s_pool.tile([P, CPG, P, KT], F16)
                    dv = dstv_sb[:, c0 * KT : (c0 + CPG) * KT]
                    in0 = bass.AP(
                        tensor=dstv_sb.tensor,
                        offset=dv.offset,
                        ap=[dv.ap[0], [KT, CPG], [0, P], [1, KT]],
                    )
                    in1 = bass.AP(
                        tensor=iotar_sb.tensor,
                        offset=iotar_sb[:].offset,
                        ap=[iotar_sb[:].ap[0], [0, CPG], [KT, P], [1, KT]],
                    )
                    nc.vector.tensor_tensor(
                        out=sel[:], in0=in0, in1=in1, op=mybir.AluOpType.is_equal
                    )
                    # segment-sum matmuls, PSUM bank per BANKG chunks
                    for b in range(CPG // BANKG):
                        do_flush(c0, b, sel, gath, is_final)

                # each piece's reduce dispatches two calls into the NEXT
                # piece so its sem waits don't head-of-line block gathers
                rs_at = {}
                acc = 0
                for pi in range(len(PIECES) - 1):
                    acc += PIECES[pi][1]
                    rs_at[N_CORES * acc // CPG + 1] = pi
                for call in range(CALLS):
                    do_call(call, l, is_final)
                    pi = rs_at.get(call)
                    if pi is not None and not is_final:
                        nc.gpsimd.collective_compute(
                            "ReduceScatter", mybir.AluOpType.add,
                            replica_groups=rg,
                            ins=[rs_ins[pi].opt()], outs=[rs_outs[pi].opt()],
                        )

                if is_final:
                    nc.gpsimd.collective_compute(
                        "ReduceScatter", mybir.AluOpType.add, replica_groups=rg,
                        ins=[rs2_in.opt()], outs=[rs2_out.opt()],
                    )
                else:
                    nc.gpsimd.collective_compute(
                        "ReduceScatter", mybir.AluOpType.add, replica_groups=rg,
                        ins=[rs_ins[-1].opt()], outs=[rs_outs[-1].opt()],
                    )

                # ---------------- dense transform on own chunks ------------
                if is_final:
                    # z4 = rs2_out (agg @ Wrel2 summed) + Wroot2^T h + b2
                    agg2_sb = agg_pool.tile([OUT, SLOTS], F16, tag="agg")
                    nc.sync.dma_start(out=agg2_sb[:], in_=rs2_out[:])
                    for zb in range((CHUNKS + ZBLK - 1) // ZBLK):
                        w = min(ZBLK * P, SLOTS - zb * ZBLK * P)
                        cs = slice(zb * ZBLK * P, zb * ZBLK * P + w)
                        ps_z = psZ.tile([OUT, ZBLK * P], F32, space="PSUM", tag="psz")
                        nc.tensor.matmul(
                            out=ps_z[:, :w], lhsT=w2_sb[:, OUT : 2 * OUT],
                            rhs=hT_prev[:, cs], start=True, stop=False,
                        )
                        nc.tensor.matmul(
                            out=ps_z[:, :w], lhsT=b2_sb[:],
                            rhs=ones_sb[:, :w], start=False, stop=True,
                        )
                        nc.vector.tensor_tensor(
                            out=out_sb[:, cs], in0=ps_z[:, :w],
                            in1=agg2_sb[:, cs], op=mybir.AluOpType.add,
                        )
                    nc.sync.dma_start(out=p_out[:], in_=out_sb[:])
                    continue

                agg_sb = agg_pool.tile([P, SLOTS], F16, tag="agg")
                for pi, (pc0, plen, _) in enumerate(PIECES):
                    nc.sync.dma_start(
                        out=agg_sb[:, pc0 * P : (pc0 + plen) * P],
                        in_=rs_outs[pi][:],
                    )

                z_all = z_pool.tile([P, SLOTS], F16, tag="z")
                stats = stat_pool.tile([P, CHUNKS, nc.vector.BN_STATS_DIM], F32)
                w_rel = wrel_sb[:, l * D : (l + 1) * D]
                w_root = wroot_sb[:, l * D : (l + 1) * D]

                def do_zblock(zb):
                    w = min(ZBLK * P, SLOTS - zb * ZBLK * P)
                    cs = slice(zb * ZBLK * P, zb * ZBLK * P + w)
                    ps_z = psZ.tile([P, ZBLK * P], F32, space="PSUM", tag="psz")
                    nc.tensor.matmul(
                        out=ps_z[:, :w], lhsT=w_rel, rhs=agg_sb[:, cs],
                        start=True, stop=False,
                    )
                    nc.tensor.matmul(
                        out=ps_z[:, :w], lhsT=w_root, rhs=hT_prev[:, cs],
                        start=False, stop=True,
                    )
                    nc.scalar.activation(
                        out=z_all[:, cs], in_=ps_z[:, :w],
                        func=mybir.ActivationFunctionType.Copy,
                    )
                    for ci in range(w // P):
                        c = zb * ZBLK + ci
                        nc.vector.bn_stats(
                            out=stats[:, c, :],
                            in_=z_all[:, c * P : (c + 1) * P],
                        )

                for zb in range((CHUNKS + ZBLK - 1) // ZBLK):
                    do_zblock(zb)

                # ---------------- BatchNorm over all nodes -----------------
                bs = bn_pool.tile([P, 16], F32)
                mv = bs[:, 0:2]
                with tc.high_priority():
                    nc.vector.bn_aggr(out=mv, in_=stats[:])
                cc_sb = bs[:, 3:5]
                with tc.high_priority():
                    nc.vector.tensor_copy(out=cc_sb[:, 0:1], in_=mv[:, 0:1])
                    nc.vector.tensor_scalar(
                        out=cc_sb[:, 1:2], in0=mv[:, 0:1], scalar1=mv[:, 0:1],
                        scalar2=mv[:, 1:2], op0=mybir.AluOpType.mult,
                        op1=mybir.AluOpType.add,
                    )
                cc_in = dram_cc.tile([P, 2], F32)
                cc_out = dram_cc.tile([P * N_CORES, 2], F32, addr_space="Shared")
                nc.sync.dma_start(out=cc_in[:], in_=cc_sb)
                nc.gpsimd.collective_compute(
                    "AllGather", mybir.AluOpType.bypass, replica_groups=rg,
                    ins=[cc_in.opt()], outs=[cc_out.opt()],
                )
                cc_all = bn_pool.tile([P, 2, N_CORES], F32)
                cc_src = bass.AP(
                    tensor=cc_out.tensor,
                    offset=cc_out[:].offset,
                    ap=[[2, P], [1, 2], [2 * P, N_CORES]],
                )
                nc.sync.dma_start(out=cc_all[:], in_=cc_src)
                cc_res = bs[:, 5:7]
                nc.vector.tensor_reduce(
                    out=cc_res.rearrange("p (a b) -> p a b", a=2),
                    in_=cc_all[:],
                    axis=mybir.AxisListType.X,
                    op=mybir.AluOpType.add,
                )
                # mu = C/8 * sum(mean_c); E2 = C/8 * sum(E2_c); var = E2 - mu^2
                mu = bs[:, 7:8]
                nc.vector.tensor_scalar(
                    out=mu, in0=cc_res[:, 0:1], scalar2=None,
                    op0=mybir.AluOpType.mult, scalar1=STATC / N_CORES,
                )
                var = bs[:, 8:9]
                nc.vector.tensor_scalar(
                    out=var, in0=cc_res[:, 1:2], scalar2=None,
                    op0=mybir.AluOpType.mult, scalar1=STATC / N_CORES,
                )
                mu2 = bs[:, 9:10]
                nc.vector.tensor_tensor(
                    out=mu2, in0=mu, in1=mu, op=mybir.AluOpType.mult
                )
                nc.vector.tensor_tensor(
                    out=var, in0=var, in1=mu2, op=mybir.AluOpType.subtract
                )
                rstd = bs[:, 10:11]
                nc.scalar.activation(
                    out=rstd, in_=var,
                    func=mybir.ActivationFunctionType.Sqrt,
                    bias=eps_sb[:], scale=1.0,
                )
                nc.vector.reciprocal(out=rstd, in_=rstd)
                scale = bs[:, 11:12]
                nc.vector.tensor_tensor(
                    out=scale, in0=rstd, in1=gammaT_sb[:, l : l + 1],
                    op=mybir.AluOpType.mult,
                )
                shift = bs[:, 12:13]
                nc.vector.tensor_tensor(
                    out=shift, in0=mu, in1=scale, op=mybir.AluOpType.mult
                )
                nc.vector.tensor_tensor(
                    out=shift, in0=betaT_sb[:, l : l + 1], in1=shift,
                    op=mybir.AluOpType.subtract,
                )

                # ---------------- BN apply + rebuild local table -----------
                hT_new = hT_pool.tile([P, SLOTS], F16, tag="hT")
                h_tab = dram_tab.tile([SLOTS, D], F16)

                def do_table_block(cb):
                    gs = slice(cb * TBLK * P, (cb + 1) * TBLK * P)
                    nc.scalar.activation(
                        out=hT_new[:, gs], in_=z_all[:, gs],
                        func=mybir.ActivationFunctionType.Relu,
                        bias=shift, scale=scale,
                    )
                    # zero pad slots: next layer's Wroot term and BN stats
                    # need exact zeros there
                    nc.vector.tensor_tensor(
                        out=hT_new[:, gs], in0=hT_new[:, gs],
                        in1=mask_sb[:, gs], op=mybir.AluOpType.mult,
                    )
                    # xbar transpose to node-major, then one table write
                    tt = t16_pool.tile([P, TBLK, P], F16)
                    nc.scalar.dma_start(out=tt[:], in_=hT_new[:, gs],
                                        transpose=True)
                    dst = bass.AP(
                        tensor=h_tab.tensor,
                        offset=h_tab[:].offset + cb * TBLK * P * D,
                        ap=[[D, P], [P * D, TBLK], [1, D]],
                    )
                    nc.sync.dma_start(out=dst, in_=tt[:])

                for cb in range(CHUNKS // TBLK):
                    do_table_block(cb)
                hT_prev = hT_new

    lower_extended_insts(nc)
    _split_multiwait(nc)
    return nc


_PROGRAM_CACHE = {}


def _get_program():
    if "p" not in _PROGRAM_CACHE:
        _PROGRAM_CACHE["p"] = build_program()
    return _PROGRAM_CACHE["p"]


def _make_in_maps(idx_cores, dstv_cores, x_loc, xT_loc, mask,
                  Wrel, Wroot, b, gamma, beta, Wrel2, Wroot2, b2):
    iotar = np.repeat(np.arange(P, dtype=np.float16), KT)[None, :].repeat(P, 0)
    w2 = np.concatenate(
        [np.asarray(Wrel2, np.float32), np.asarray(Wroot2, np.float32)], axis=1
    )
    common = dict(
        wrel=np.ascontiguousarray(np.asarray(Wrel, np.float16)),
        wroot=np.ascontiguousarray(np.asarray(Wroot, np.float16)),
        w2=np.ascontiguousarray(w2.astype(np.float16)),
        gammaT=np.ascontiguousarray(np.asarray(gamma, np.float32).T),
        betaT=np.ascontiguousarray(np.asarray(beta, np.float32).T),
        b2r=np.asarray(b2, np.float16).reshape(1, OUT),
        iotar=np.ascontiguousarray(iotar),
    )
    in_maps = []
    for c in range(N_CORES):
        m = dict(common)
        m["x_loc"] = x_loc[c]
        m["xT_loc"] = xT_loc[c]
        m["gidx"] = idx_cores[c]
        m["dstv"] = dstv_cores[c]
        m["mask16"] = np.ascontiguousarray(mask[c])
        in_maps.append(m)
    return in_maps


def run(x, edge_index, Wrel, Wroot, b, gamma, beta, Wrel2, Wroot2, b2):
    """Returns (output [N, OUT] float32, nc) — nc exposed for profiling.

    Note: inner-layer GraphConv biases `b` are mathematically absorbed by
    training-mode BatchNorm and intentionally unused.
    """
    newid, idx_cores, dstv_cores, x_loc, xT_loc, mask = _preprocess(x, edge_index)
    nc = _get_program()
    in_maps = _make_in_maps(
        idx_cores, dstv_cores, x_loc, xT_loc, mask,
        Wrel, Wroot, b, gamma, beta, Wrel2, Wroot2, b2,
    )
    from concourse.bass_utils import run_bass_kernel_spmd

    res = run_bass_kernel_spmd(nc, in_maps, list(range(N_CORES)))
    full = np.concatenate(
        [res.results[c]["z4T"].T for c in range(N_CORES)], axis=0
    )  # [N_PAD, OUT]
    return full[newid].astype(np.float32), nc


def kernel(**inputs):
    out, _ = run(**{k: np.asarray(v) for k, v in inputs.items()})
    return out
